# revision 27
# baseline (speedup 1.0000x reference)
"""CQAttention (BiDAF context-query attention) forward kernel for 8 Trainium2
NeuronCores — bf16 edition.

Full inputs: context (64,128,1024) f32, question (64,128,128) f32, w (384,) f32.
Full output: (64, 512, 1024) f32.

Sharding: pure data parallel over batch — 8 batches per core, w replicated.
The 2e-2 relative-error gate leaves ample room for bf16 (host emulation of the
full bf16 pipeline measures ~1.0e-3), which halves DMA bytes — the roofline
resource — and doubles DVE throughput on 16-bit ops.

Per batch (X = context[b] (H,C) bf16, Y = question[b] (H,Q) bf16):
    Z    = wcq*Y + wc                  (H,Q)
    S'_c = X_c^T @ Z   (8 chunks)      (C,Q)  -> P' = exp(S') bf16 (SBUF direct)
    tt   = sum_c P'_c-contract XT_c    (Q,H+1): XT carries a host-injected ones
           column, so tt[:,128] accumulates d = colsum(P') — the softmax
           denominators come out of the tt matmul for free.
    P    = P'^T  via 8 PE transposes (bf16 PSUM -> 2x-mode DVE/ACT copies)
    A    = (r*Y^T)^T @ P               (H,C)  = a^T
    Bm   = (r^2*tt)^T @ P              (H,C)  = b^T
    out  = [A; X*A; X*Bm]  (3H,C) bf16; block 0 (= context) is assembled
           host-side as a pure input passthrough.

X^T and Y^T are supplied by the host in an SBUF-tiled layout
(xt[b,p,c,h] = X[h,128c+p]) so their DMAs are plain contiguous 2KB-per-
partition transfers — the on-chip alternatives (DMA xbar transpose: 387B
packets; PE transposes: PSUM round-trips on the busiest engines) both lose.

DMA plumbing (v2): each DMA engine caps at ~23GB/s regardless of packet
size, so the 16-engine aggregate (~375GB/s) is the roofline and the win is
keeping all engines continuously fed with few, large descriptors: inputs
move as batch PAIRS (9.3KB/partition rows, 4 descriptors total), outputs as
one 6KB-row descriptor per batch into a (b, partition, [A|XA|XB]) DRAM
layout that the host untangles.
"""

import os
import sys

import numpy as np

if "/opt/trn_rl_repo" not in sys.path:
    sys.path.insert(0, "/opt/trn_rl_repo")

B, H, C, Q = 64, 128, 1024, 128
NCORES = 8
BPC = B // NCORES  # batches per core
NPAIR = BPC // 2  # input DMAs move two batches at once (9.3KB rows)
XTW = 132  # X^T chunk width: 128 data + ones col + pad
# packed input layout (per batch, per partition): [X | XT(8 chunks) | Y | YT]
OFF_XT = C
OFF_Y = C + 8 * XTW
OFF_YT = OFF_Y + Q
IN_W = OFF_YT + H


def _ensure_ntff_hook():
    """This container's `antenv` stub lacks `axon_hooks`, which
    bass_utils needs for NTFF profiling under axon (trace=True). Install
    a functional shadow module + register the ctypes-based hook."""
    import types

    try:
        from antenv.axon_hooks import get_axon_ntff_profile_hook  # noqa: F401

        return  # real module present
    except ImportError:
        pass
    try:
        import antenv

        mod = types.ModuleType("antenv.axon_hooks")
        _state = {"hook": None}

        def set_axon_ntff_profile_hook(h):
            _state["hook"] = h

        def get_axon_ntff_profile_hook():
            return _state["hook"]

        mod.set_axon_ntff_profile_hook = set_axon_ntff_profile_hook
        mod.get_axon_ntff_profile_hook = get_axon_ntff_profile_hook
        sys.modules["antenv.axon_hooks"] = mod
        antenv.axon_hooks = mod

        from trn_agent_boot.trn_boot import _ntff_profile_via_ctypes

        set_axon_ntff_profile_hook(
            _ntff_profile_via_ctypes("/opt/axon/libaxon_pjrt.so")
        )
    except Exception:
        pass  # profiling degrades; compute still works


_ensure_ntff_hook()

LAST_RESULTS = None
_NC = None


def _build():
    from contextlib import ExitStack

    import concourse.bacc as bacc
    import concourse.mybir as mybir
    import concourse.tile as tile
    from concourse import masks

    f32 = mybir.dt.float32
    f32r = mybir.dt.float32r
    bf16 = mybir.dt.bfloat16
    EXP = mybir.ActivationFunctionType.Exp

    nc = bacc.Bacc(
        "TRN2", target_bir_lowering=False, debug=False, enable_asserts=False
    )
    in_t = nc.dram_tensor(
        "inall", (NPAIR, 128, 2 * IN_W), bf16, kind="ExternalInput"
    ).ap()
    w_t = nc.dram_tensor("w", (3 * H,), f32, kind="ExternalInput").ap()
    out_t = nc.dram_tensor("out", (BPC, 128, 3 * C), bf16, kind="ExternalOutput").ap()

    with tile.TileContext(nc) as tc, ExitStack() as ctx:
        const = ctx.enter_context(tc.tile_pool(name="const", bufs=1))
        sb = ctx.enter_context(tc.tile_pool(name="sb", bufs=6))
        sbx = ctx.enter_context(tc.tile_pool(name="sbx", bufs=NPAIR))
        # PSUM budget (8 banks): S' f32 as 2x[128,512] double-buffered =
        # 2 banks (so next iteration's S' matmuls never WAR-wait on this
        # iteration's exp); A and B f32 [128,1024] single-buffered = 4 banks
        # (their readers run a full iteration before the next writer);
        # transpose staging bf16 = 1 bank; tt = 1 bank.
        psS = ctx.enter_context(tc.tile_pool(name="psS", bufs=2, space="PSUM"))
        psA = ctx.enter_context(tc.tile_pool(name="psA", bufs=1, space="PSUM"))
        psB = ctx.enter_context(tc.tile_pool(name="psB", bufs=1, space="PSUM"))
        psb = ctx.enter_context(tc.tile_pool(name="psb", bufs=1, space="PSUM"))
        pstt = ctx.enter_context(tc.tile_pool(name="pstt", bufs=1, space="PSUM"))

        state = {}  # keyed by batch index -> dict of live tiles

        def stage0(p, eng, split=False):
            # one packed input DMA per batch PAIR: 9.3KB/partition contiguous
            # rows keep every DMA engine at its ~23GB/s cap with minimal
            # queue-issue cost (each dma_start burns ~0.7us of queue time).
            # The first pair is split into pieces (Y0+YT0 | X0+XT0 | batch1)
            # so the Z -> S' chain can start after the first two pieces land.
            IN = sbx.tile([128, 2 * IN_W], bf16, tag="IN")
            if split:
                eng.dma_start(IN[:, OFF_Y:IN_W], in_t[p, :, OFF_Y:IN_W])
                eng.dma_start(IN[:, 0:OFF_Y], in_t[p, :, 0:OFF_Y])
                nc.gpsimd.dma_start(
                    IN[:, IN_W : 2 * IN_W], in_t[p, :, IN_W : 2 * IN_W]
                )
            else:
                eng.dma_start(IN[:], in_t[p])
            for h in range(2):
                o = h * IN_W
                state[2 * p + h] = dict(
                    XT=IN[:, o + OFF_XT : o + OFF_XT + 8 * XTW],
                    X=IN[:, o : o + C],
                    Y=IN[:, o + OFF_Y : o + OFF_Y + Q],
                    YT=IN[:, o + OFF_YT : o + OFF_YT + H],
                )

        # w first (tiny, must not queue behind bulk input transfers), then
        # the first two pairs; pair0's second batch rides the gpsimd queue so
        # DMA round-robin fairness doesn't starve batch 0's pieces.
        w_row = const.tile([1, 3 * H], f32r, tag="w_row")
        nc.sync.dma_start(w_row[:], w_t.unsqueeze(0).bitcast(f32r))
        stage0(0, nc.sync, split=True)
        if NPAIR > 1:
            stage0(1, nc.sync)

        ident = const.tile([128, 128], f32, tag="ident")
        masks.make_identity(nc, ident[:])
        identr = const.tile([128, 128], f32r, tag="identr")
        nc.vector.tensor_copy(identr[:], ident[:])
        identb = const.tile([128, 128], bf16, tag="identb")
        nc.vector.tensor_copy(identb[:], ident[:])

        # the (128,1) w columns are produced by K=1 PE matmuls against
        # identity.
        wc = const.tile([128, 1], f32, tag="wc")
        wcq = const.tile([128, 1], f32, tag="wcq")

        def stage1(b):
            st = state[b]
            Y = st["Y"]

            if b == 0:
                wps = psS.tile([128, 512], f32, tag="psS")
                nc.tensor.matmul(
                    wps[:, 0:128],
                    w_row[0:1, H : 2 * H],
                    identr[0:1, 0:128],
                    start=True,
                    stop=True,
                )
                nc.tensor.matmul(
                    wps[:, 128:256],
                    w_row[0:1, 2 * H : 3 * H],
                    identr[0:1, 0:128],
                    start=True,
                    stop=True,
                )
                nc.vector.tensor_copy(wc[:], wps[:, 0:1])
                nc.vector.tensor_copy(wcq[:], wps[:, 128:129])

            # Z = wcq * Y + wc on Pool (SBUF-only; Pool is otherwise idle)
            Z = sb.tile([H, Q], bf16, tag="Z")
            nc.gpsimd.tensor_scalar(
                Z[:],
                Y[:],
                wcq[:],
                wc[:],
                mybir.AluOpType.mult,
                mybir.AluOpType.add,
            )
            st.update(Z=Z)

        def sprime_mms(b):
            # S'^T = Z^T X in (Q, C) layout: ONE stationary load (Z) and two
            # 512-col matmuls instead of eight 128-col ones; exp yields
            # P = exp(S')^T directly in the layout the A/B matmuls consume.
            # Each half goes exp'd as soon as its matmul lands.
            st = state[b]
            X, Z = st["X"], st["Z"]
            P = sb.tile([Q, C], bf16, tag="P")
            for j in range(2):
                Sp = psS.tile([128, 512], f32, tag="psS")
                nc.tensor.matmul(
                    Sp[:],
                    Z[:],
                    X[:, j * 512 : (j + 1) * 512],
                    start=True,
                    stop=True,
                )
                nc.scalar.activation(P[:, j * 512 : (j + 1) * 512], Sp[:], EXP)
            st.update(P=P)

        def ab_mms(b):
            # old batch's A/B matmuls + PSUM consumers: all inputs ready,
            # so these go early in every engine queue
            st = state[b]
            P, YTs, tts = st["P"], st["YTs"], st["tts"]
            OUT = sb.tile([H, 3 * C], bf16, tag="OUT")
            Aps = psA.tile([H, 1024], f32, tag="psA")
            for j in range(2):
                nc.tensor.matmul(
                    Aps[:, j * 512 : (j + 1) * 512],
                    YTs[:],
                    P[:, j * 512 : (j + 1) * 512],
                    start=True,
                    stop=True,
                )
            nc.scalar.copy(OUT[:, 0:C], Aps[:])
            Bps = psB.tile([H, 1024], f32, tag="psB")
            for j in range(2):
                nc.tensor.matmul(
                    Bps[:, j * 512 : (j + 1) * 512],
                    tts[:],
                    P[:, j * 512 : (j + 1) * 512],
                    start=True,
                    stop=True,
                )
            st.update(OUT=OUT, Bps=Bps)

        def muls_out(b):
            st = state[b]
            X, OUT, Bps = st["X"], st["OUT"], st["Bps"]
            # X*B straight from PSUM (B itself is never output); halves keep
            # the DVE op overhead low for the f32-source (1x mode) reads
            for j in range(2):
                nc.vector.tensor_mul(
                    OUT[:, 2 * C + j * 512 : 2 * C + (j + 1) * 512],
                    X[:, j * 512 : (j + 1) * 512],
                    Bps[:, j * 512 : (j + 1) * 512],
                )
            # X*A all-bf16 (2x DVE mode), one wide op
            nc.vector.tensor_mul(OUT[:, C : 2 * C], X[:], OUT[:, 0:C])
            # one 6KB-row DMA ships the whole batch: out DRAM layout is
            # (b, partition, [A|XA|XB]) and the host untangles the blocks.
            # Tail batches ride the sync queue (input issues have ceased).
            eng = nc.sync if b >= BPC - 2 else nc.gpsimd
            eng.dma_start(out_t[b], OUT[:])

        def ptr(b):
            st = state[b]
            P = st["P"]
            # PT = P^T = P' via PE transposes (bf16 PSUM), 2x-mode DVE copies.
            # Runs one iteration BEFORE the tt matmuls that consume PT, so
            # the PE never stalls on the transpose -> copy chain.
            Pp = psb.tile([128, 1024], bf16, tag="ptp")
            PT = sb.tile([128, C], bf16, tag="PT")
            for k in range(8):
                nc.tensor.transpose(
                    Pp[:, k * 128 : (k + 1) * 128],
                    P[:, k * 128 : (k + 1) * 128],
                    identb[:],
                )
            nc.vector.tensor_copy(PT[:], Pp[:])
            st.update(PT=PT)

        def tt_mms(b):
            st = state[b]
            XT, YT, PT = st["XT"], st["YT"], st["PT"]
            # tt = P' contracted with X^T chunks (Q,H); col 128 accumulates
            # d = colsum(P') via the host-injected ones column in XT
            tt = pstt.tile([Q, XTW], f32, tag="tt")
            for c in range(8):
                nc.tensor.matmul(
                    tt[:],
                    PT[:, c * 128 : (c + 1) * 128],
                    XT[:, c * XTW : (c + 1) * XTW],
                    start=(c == 0),
                    stop=(c == 7),
                )
            # softmax denominators out of tt's ones column
            rr = sb.tile([Q, 1], f32, tag="rr")
            nc.vector.reciprocal(rr[:], tt[:, 128:129])
            r2 = sb.tile([Q, 1], f32, tag="r2")
            nc.gpsimd.tensor_mul(r2[:], rr[:], rr[:])
            # YTs/r2 on GpSimd (SBUF-only, and GpSimd is nearly idle);
            # tts stays on ACT (GpSimd has no PSUM port)
            YTs = sb.tile([Q, H], bf16, tag="YTs")
            nc.gpsimd.tensor_scalar_mul(YTs[:], YT[:], rr[:])
            tts = sb.tile([Q, H], bf16, tag="tts")
            nc.scalar.mul(tts[:], tt[:, 0:128], r2[:])
            st.update(YTs=YTs, tts=tts)

        # 4-deep software pipeline; per-engine queue order is chosen so every
        # op's inputs are either >= 1 iteration old or produced earlier in
        # the same iteration by an engine that runs ahead of the consumer:
        #   PE:  S'(b-1) | A/B(b-3) | tt(b-2) | transposes(b-1, after exp)
        #   ACT: exp(b-1) | A-copy(b-3) | tts(b-2)
        #   DVE: XB/XA(b-3) | scales(b-2) | PT copies(b-1, after transposes)
        # Input pairs are paced every other iteration (2 pairs ahead of use).
        for it in range(BPC + 3):
            b1, b2, b3, b4 = it, it - 1, it - 2, it - 3
            p = it // 2 + 2
            if it % 2 == 0 and p < NPAIR:
                stage0(p, nc.sync)
            if b1 < BPC:
                stage1(b1)
            if 0 <= b2 < BPC:
                sprime_mms(b2)
            if 0 <= b4:
                ab_mms(b4)
                muls_out(b4)
            if 0 <= b3 < BPC:
                tt_mms(b3)
            if 0 <= b2 < BPC:
                ptr(b2)
            if 0 <= b4:
                del state[b4]

    nc.compile()
    return nc


def kernel(context, question, w):
    global _NC, LAST_RESULTS
    import ml_dtypes
    from concourse import bass_utils

    if _NC is None:
        _NC = _build()

    bf16 = ml_dtypes.bfloat16
    context = np.asarray(context)
    question = np.asarray(question)
    ctx16 = np.ascontiguousarray(context.astype(bf16))
    q16 = np.ascontiguousarray(question.astype(bf16))
    w = np.ascontiguousarray(np.asarray(w), dtype=np.float32)

    # packed per-batch input: [X | XT tiled (xt[b,p,c,h]=X[b,h,128c+p], ones
    # col at 128) | Y | YT]; batches are then paired so each input DMA moves
    # one contiguous 9.3KB/partition row.
    inall = np.zeros((B, 128, IN_W), dtype=bf16)
    inall[:, :, 0:C] = ctx16
    xt = inall[:, :, OFF_XT : OFF_XT + 8 * XTW].reshape(B, 128, 8, XTW)
    xt[..., 0:128] = (
        ctx16.transpose(0, 2, 1).reshape(B, 8, 128, H).transpose(0, 2, 1, 3)
    )
    xt[..., 128] = np.asarray(1.0, dtype=bf16)
    inall[:, :, OFF_Y : OFF_Y + Q] = q16
    inall[:, :, OFF_YT : OFF_YT + H] = q16.transpose(0, 2, 1)
    in2 = np.ascontiguousarray(
        inall.reshape(B // 2, 2, 128, IN_W).transpose(0, 2, 1, 3)
    ).reshape(B // 2, 128, 2 * IN_W)

    in_maps = [
        {
            "inall": in2[c * NPAIR : (c + 1) * NPAIR],
            "w": w,
        }
        for c in range(NCORES)
    ]
    trace = bool(int(os.environ.get("KTRACE", "0")))
    LAST_RESULTS = bass_utils.run_bass_kernel_spmd(
        _NC, in_maps, core_ids=list(range(NCORES)), trace=trace
    )
    out = np.empty((B, 4 * H, C), dtype=np.float32)
    out[:, 0:H, :] = np.asarray(context, dtype=np.float32)
    for c in range(NCORES):
        # device out layout: (b, partition=h, [A|XA|XB] by 1024-col blocks)
        res = LAST_RESULTS.results[c]["out"].reshape(BPC, 128, 3, C)
        out[c * BPC : (c + 1) * BPC, H:, :] = (
            res.transpose(0, 2, 1, 3).reshape(BPC, 3 * H, C).astype(np.float32)
        )
    return out



# revision 28
# speedup vs baseline: 1.1973x; 1.1973x over previous
"""CQAttention (BiDAF context-query attention) forward kernel for 8 Trainium2
NeuronCores — bf16 edition.

Full inputs: context (64,128,1024) f32, question (64,128,128) f32, w (384,) f32.
Full output: (64, 512, 1024) f32.

Sharding: pure data parallel over batch — 8 batches per core, w replicated.
The 2e-2 relative-error gate leaves ample room for bf16 (host emulation of the
full bf16 pipeline measures ~1.0e-3), which halves DMA bytes — the roofline
resource — and doubles DVE throughput on 16-bit ops.

Per batch (X = context[b] (H,C) bf16, Y = question[b] (H,Q) bf16):
    Z    = wcq*Y + wc                  (H,Q)
    S'_c = X_c^T @ Z   (8 chunks)      (C,Q)  -> P' = exp(S') bf16 (SBUF direct)
    tt   = sum_c P'_c-contract XT_c    (Q,H+1): XT carries a host-injected ones
           column, so tt[:,128] accumulates d = colsum(P') — the softmax
           denominators come out of the tt matmul for free.
    P    = P'^T  via 8 PE transposes (bf16 PSUM -> 2x-mode DVE/ACT copies)
    A    = (r*Y^T)^T @ P               (H,C)  = a^T
    Bm   = (r^2*tt)^T @ P              (H,C)  = b^T
    out  = [A; X*A; X*Bm]  (3H,C) bf16; block 0 (= context) is assembled
           host-side as a pure input passthrough.

X^T and Y^T are supplied by the host in an SBUF-tiled layout
(xt[b,p,c,h] = X[h,128c+p]) so their DMAs are plain contiguous 2KB-per-
partition transfers — the on-chip alternatives (DMA xbar transpose: 387B
packets; PE transposes: PSUM round-trips on the busiest engines) both lose.

DMA plumbing (v2): each DMA engine caps at ~23GB/s regardless of packet
size, so the 16-engine aggregate (~375GB/s) is the roofline and the win is
keeping all engines continuously fed with few, large descriptors: inputs
move as batch PAIRS (9.3KB/partition rows, 4 descriptors total), outputs as
one 6KB-row descriptor per batch into a (b, partition, [A|XA|XB]) DRAM
layout that the host untangles.
"""

import os
import sys

import numpy as np

if "/opt/trn_rl_repo" not in sys.path:
    sys.path.insert(0, "/opt/trn_rl_repo")

B, H, C, Q = 64, 128, 1024, 128
NCORES = 8
BPC = B // NCORES  # batches per core
NPAIR = BPC // 2  # input DMAs move two batches at once (9.3KB rows)
XTW = 132  # X^T chunk width: 128 data + ones col + pad
# packed input layout (per batch, per partition): [X | XT(8 chunks) | Y | YT]
OFF_XT = C
OFF_Y = C + 8 * XTW
OFF_YT = OFF_Y + Q
IN_W = OFF_YT + H


def _ensure_ntff_hook():
    """This container's `antenv` stub lacks `axon_hooks`, which
    bass_utils needs for NTFF profiling under axon (trace=True). Install
    a functional shadow module + register the ctypes-based hook."""
    import types

    try:
        from antenv.axon_hooks import get_axon_ntff_profile_hook  # noqa: F401

        return  # real module present
    except ImportError:
        pass
    try:
        import antenv

        mod = types.ModuleType("antenv.axon_hooks")
        _state = {"hook": None}

        def set_axon_ntff_profile_hook(h):
            _state["hook"] = h

        def get_axon_ntff_profile_hook():
            return _state["hook"]

        mod.set_axon_ntff_profile_hook = set_axon_ntff_profile_hook
        mod.get_axon_ntff_profile_hook = get_axon_ntff_profile_hook
        sys.modules["antenv.axon_hooks"] = mod
        antenv.axon_hooks = mod

        from trn_agent_boot.trn_boot import _ntff_profile_via_ctypes

        set_axon_ntff_profile_hook(
            _ntff_profile_via_ctypes("/opt/axon/libaxon_pjrt.so")
        )
    except Exception:
        pass  # profiling degrades; compute still works


_ensure_ntff_hook()

LAST_RESULTS = None
_NC = None


def _build():
    from contextlib import ExitStack

    import concourse.bacc as bacc
    import concourse.mybir as mybir
    import concourse.tile as tile
    from concourse import masks

    f32 = mybir.dt.float32
    f32r = mybir.dt.float32r
    bf16 = mybir.dt.bfloat16
    EXP = mybir.ActivationFunctionType.Exp

    nc = bacc.Bacc(
        "TRN2", target_bir_lowering=False, debug=False, enable_asserts=False
    )
    in_t = nc.dram_tensor(
        "inall", (NPAIR, 128, 2 * IN_W), bf16, kind="ExternalInput"
    ).ap()
    w_t = nc.dram_tensor("w", (3 * H,), f32, kind="ExternalInput").ap()
    out_t = nc.dram_tensor("out", (BPC, 128, 3 * C), bf16, kind="ExternalOutput").ap()

    with tile.TileContext(nc) as tc, ExitStack() as ctx:
        const = ctx.enter_context(tc.tile_pool(name="const", bufs=1))
        sb = ctx.enter_context(tc.tile_pool(name="sb", bufs=6))
        sbx = ctx.enter_context(tc.tile_pool(name="sbx", bufs=NPAIR))
        # PSUM budget (8 banks): S' f32 as 2x[128,512] double-buffered =
        # 2 banks (so next iteration's S' matmuls never WAR-wait on this
        # iteration's exp); A and B f32 [128,1024] single-buffered = 4 banks
        # (their readers run a full iteration before the next writer);
        # transpose staging bf16 = 1 bank; tt = 1 bank.
        psS = ctx.enter_context(tc.tile_pool(name="psS", bufs=2, space="PSUM"))
        psA = ctx.enter_context(tc.tile_pool(name="psA", bufs=1, space="PSUM"))
        psB = ctx.enter_context(tc.tile_pool(name="psB", bufs=1, space="PSUM"))
        psb = ctx.enter_context(tc.tile_pool(name="psb", bufs=1, space="PSUM"))
        pstt = ctx.enter_context(tc.tile_pool(name="pstt", bufs=1, space="PSUM"))

        state = {}  # keyed by batch index -> dict of live tiles

        def stage0(p, eng, split=False):
            # one packed input DMA per batch PAIR: 9.3KB/partition contiguous
            # rows keep every DMA engine at its ~23GB/s cap with minimal
            # queue-issue cost (each dma_start burns ~0.7us of queue time).
            # The first pair is split into pieces (Y0+YT0 | X0+XT0 | batch1)
            # so the Z -> S' chain can start after the first two pieces land.
            IN = sbx.tile([128, 2 * IN_W], bf16, tag="IN")
            if split:
                eng.dma_start(IN[:, OFF_Y:IN_W], in_t[p, :, OFF_Y:IN_W])
                eng.dma_start(IN[:, 0:OFF_Y], in_t[p, :, 0:OFF_Y])
                nc.gpsimd.dma_start(
                    IN[:, IN_W : 2 * IN_W], in_t[p, :, IN_W : 2 * IN_W]
                )
            else:
                eng.dma_start(IN[:], in_t[p])
            for h in range(2):
                o = h * IN_W
                state[2 * p + h] = dict(
                    XT=IN[:, o + OFF_XT : o + OFF_XT + 8 * XTW],
                    X=IN[:, o : o + C],
                    Y=IN[:, o + OFF_Y : o + OFF_Y + Q],
                    YT=IN[:, o + OFF_YT : o + OFF_YT + H],
                )

        # w first (tiny, must not queue behind bulk input transfers), then
        # the first two pairs; pair0's second batch rides the gpsimd queue so
        # DMA round-robin fairness doesn't starve batch 0's pieces.
        w_row = const.tile([1, 3 * H], f32r, tag="w_row")
        nc.sync.dma_start(w_row[:], w_t.unsqueeze(0).bitcast(f32r))
        stage0(0, nc.sync, split=True)
        if NPAIR > 1:
            stage0(1, nc.sync)

        ident = const.tile([128, 128], f32, tag="ident")
        masks.make_identity(nc, ident[:])
        identr = const.tile([128, 128], f32r, tag="identr")
        nc.vector.tensor_copy(identr[:], ident[:])
        identb = const.tile([128, 128], bf16, tag="identb")
        nc.vector.tensor_copy(identb[:], ident[:])

        # the (128,1) w columns are produced by K=1 PE matmuls against
        # identity.
        wc = const.tile([128, 1], f32, tag="wc")
        wcq = const.tile([128, 1], f32, tag="wcq")

        def stage1(b):
            st = state[b]
            Y = st["Y"]

            if b == 0:
                wps = psS.tile([128, 512], f32, tag="psS")
                nc.tensor.matmul(
                    wps[:, 0:128],
                    w_row[0:1, H : 2 * H],
                    identr[0:1, 0:128],
                    start=True,
                    stop=True,
                )
                nc.tensor.matmul(
                    wps[:, 128:256],
                    w_row[0:1, 2 * H : 3 * H],
                    identr[0:1, 0:128],
                    start=True,
                    stop=True,
                )
                nc.vector.tensor_copy(wc[:], wps[:, 0:1])
                nc.vector.tensor_copy(wcq[:], wps[:, 128:129])

            # Z = wcq * Y + wc on Pool (SBUF-only; Pool is otherwise idle)
            Z = sb.tile([H, Q], bf16, tag="Z")
            nc.gpsimd.tensor_scalar(
                Z[:],
                Y[:],
                wcq[:],
                wc[:],
                mybir.AluOpType.mult,
                mybir.AluOpType.add,
            )
            st.update(Z=Z)

        def sprime_mms(b):
            # S'^T = Z^T X in (Q, C) layout: ONE stationary load (Z) and two
            # 512-col matmuls instead of eight 128-col ones; exp yields
            # P = exp(S')^T directly in the layout the A/B matmuls consume.
            # Each half goes exp'd as soon as its matmul lands.
            st = state[b]
            X, Z = st["X"], st["Z"]
            P = sb.tile([Q, C], bf16, tag="P")
            for j in range(2):
                Sp = psS.tile([128, 512], f32, tag="psS")
                nc.tensor.matmul(
                    Sp[:],
                    Z[:],
                    X[:, j * 512 : (j + 1) * 512],
                    start=True,
                    stop=True,
                )
                nc.scalar.activation(P[:, j * 512 : (j + 1) * 512], Sp[:], EXP)
            st.update(P=P)

        def ab_mms(b):
            # old batch's A/B matmuls + PSUM consumers: all inputs ready,
            # so these go early in every engine queue
            st = state[b]
            P, YTs, tts = st["P"], st["YTs"], st["tts"]
            OUT = sb.tile([H, 3 * C], bf16, tag="OUT")
            Aps = psA.tile([H, 1024], f32, tag="psA")
            for j in range(2):
                nc.tensor.matmul(
                    Aps[:, j * 512 : (j + 1) * 512],
                    YTs[:],
                    P[:, j * 512 : (j + 1) * 512],
                    start=True,
                    stop=True,
                )
            nc.scalar.copy(OUT[:, 0:C], Aps[:])
            Bps = psB.tile([H, 1024], f32, tag="psB")
            for j in range(2):
                nc.tensor.matmul(
                    Bps[:, j * 512 : (j + 1) * 512],
                    tts[:],
                    P[:, j * 512 : (j + 1) * 512],
                    start=True,
                    stop=True,
                )
            st.update(OUT=OUT, Bps=Bps)

        def muls_out(b):
            st = state[b]
            X, OUT, Bps = st["X"], st["OUT"], st["Bps"]
            # X*B straight from PSUM (B itself is never output); halves keep
            # the DVE op overhead low for the f32-source (1x mode) reads
            for j in range(2):
                nc.vector.tensor_mul(
                    OUT[:, 2 * C + j * 512 : 2 * C + (j + 1) * 512],
                    X[:, j * 512 : (j + 1) * 512],
                    Bps[:, j * 512 : (j + 1) * 512],
                )
            # X*A all-bf16 (2x DVE mode), one wide op
            nc.vector.tensor_mul(OUT[:, C : 2 * C], X[:], OUT[:, 0:C])
            # one 6KB-row DMA ships the whole batch: out DRAM layout is
            # (b, partition, [A|XA|XB]) and the host untangles the blocks.
            # Tail batches ride the sync queue (input issues have ceased).
            eng = nc.sync if b >= BPC - 2 else nc.gpsimd
            eng.dma_start(out_t[b], OUT[:])

        def ptr(b):
            st = state[b]
            P = st["P"]
            # PT = P^T = P' via PE transposes (bf16 PSUM), 2x-mode DVE copies.
            # Runs one iteration BEFORE the tt matmuls that consume PT, so
            # the PE never stalls on the transpose -> copy chain.
            Pp = psb.tile([128, 1024], bf16, tag="ptp")
            PT = sb.tile([128, C], bf16, tag="PT")
            for k in range(8):
                nc.tensor.transpose(
                    Pp[:, k * 128 : (k + 1) * 128],
                    P[:, k * 128 : (k + 1) * 128],
                    identb[:],
                )
            nc.vector.tensor_copy(PT[:], Pp[:])
            st.update(PT=PT)

        def tt_mms(b):
            st = state[b]
            XT, YT, PT = st["XT"], st["YT"], st["PT"]
            # tt = P' contracted with X^T chunks (Q,H); col 128 accumulates
            # d = colsum(P') via the host-injected ones column in XT
            tt = pstt.tile([Q, XTW], f32, tag="tt")
            for c in range(8):
                nc.tensor.matmul(
                    tt[:],
                    PT[:, c * 128 : (c + 1) * 128],
                    XT[:, c * XTW : (c + 1) * XTW],
                    start=(c == 0),
                    stop=(c == 7),
                )
            # softmax denominators out of tt's ones column
            rr = sb.tile([Q, 1], f32, tag="rr")
            nc.vector.reciprocal(rr[:], tt[:, 128:129])
            r2 = sb.tile([Q, 1], f32, tag="r2")
            nc.vector.tensor_mul(r2[:], rr[:], rr[:])
            # YTs on DVE (GpSimd's vector-scalar path is ~8x slower);
            # tts stays on ACT (GpSimd has no PSUM port)
            YTs = sb.tile([Q, H], bf16, tag="YTs")
            nc.vector.tensor_scalar_mul(YTs[:], YT[:], rr[:])
            tts = sb.tile([Q, H], bf16, tag="tts")
            nc.scalar.mul(tts[:], tt[:, 0:128], r2[:])
            st.update(YTs=YTs, tts=tts)

        # 4-deep software pipeline; per-engine queue order is chosen so every
        # op's inputs are either >= 1 iteration old or produced earlier in
        # the same iteration by an engine that runs ahead of the consumer:
        #   PE:  S'(b-1) | A/B(b-3) | tt(b-2) | transposes(b-1, after exp)
        #   ACT: exp(b-1) | A-copy(b-3) | tts(b-2)
        #   DVE: XB/XA(b-3) | scales(b-2) | PT copies(b-1, after transposes)
        # Input pairs are paced every other iteration (2 pairs ahead of use).
        for it in range(BPC + 3):
            b1, b2, b3, b4 = it, it - 1, it - 2, it - 3
            p = it // 2 + 2
            if it % 2 == 0 and p < NPAIR:
                stage0(p, nc.sync)
            if b1 < BPC:
                stage1(b1)
            if 0 <= b2 < BPC:
                sprime_mms(b2)
            if 0 <= b4:
                ab_mms(b4)
                muls_out(b4)
            if 0 <= b3 < BPC:
                tt_mms(b3)
            if 0 <= b2 < BPC:
                ptr(b2)
            if 0 <= b4:
                del state[b4]

    nc.compile()
    return nc


def kernel(context, question, w):
    global _NC, LAST_RESULTS
    import ml_dtypes
    from concourse import bass_utils

    if _NC is None:
        _NC = _build()

    bf16 = ml_dtypes.bfloat16
    context = np.asarray(context)
    question = np.asarray(question)
    ctx16 = np.ascontiguousarray(context.astype(bf16))
    q16 = np.ascontiguousarray(question.astype(bf16))
    w = np.ascontiguousarray(np.asarray(w), dtype=np.float32)

    # packed per-batch input: [X | XT tiled (xt[b,p,c,h]=X[b,h,128c+p], ones
    # col at 128) | Y | YT]; batches are then paired so each input DMA moves
    # one contiguous 9.3KB/partition row.
    inall = np.zeros((B, 128, IN_W), dtype=bf16)
    inall[:, :, 0:C] = ctx16
    xt = inall[:, :, OFF_XT : OFF_XT + 8 * XTW].reshape(B, 128, 8, XTW)
    xt[..., 0:128] = (
        ctx16.transpose(0, 2, 1).reshape(B, 8, 128, H).transpose(0, 2, 1, 3)
    )
    xt[..., 128] = np.asarray(1.0, dtype=bf16)
    inall[:, :, OFF_Y : OFF_Y + Q] = q16
    inall[:, :, OFF_YT : OFF_YT + H] = q16.transpose(0, 2, 1)
    in2 = np.ascontiguousarray(
        inall.reshape(B // 2, 2, 128, IN_W).transpose(0, 2, 1, 3)
    ).reshape(B // 2, 128, 2 * IN_W)

    in_maps = [
        {
            "inall": in2[c * NPAIR : (c + 1) * NPAIR],
            "w": w,
        }
        for c in range(NCORES)
    ]
    trace = bool(int(os.environ.get("KTRACE", "0")))
    LAST_RESULTS = bass_utils.run_bass_kernel_spmd(
        _NC, in_maps, core_ids=list(range(NCORES)), trace=trace
    )
    out = np.empty((B, 4 * H, C), dtype=np.float32)
    out[:, 0:H, :] = np.asarray(context, dtype=np.float32)
    for c in range(NCORES):
        # device out layout: (b, partition=h, [A|XA|XB] by 1024-col blocks)
        res = LAST_RESULTS.results[c]["out"].reshape(BPC, 128, 3, C)
        out[c * BPC : (c + 1) * BPC, H:, :] = (
            res.transpose(0, 2, 1, 3).reshape(BPC, 3 * H, C).astype(np.float32)
        )
    return out

